# revision 13
# baseline (speedup 1.0000x reference)
"""GCN layer (gather -> normalize -> scatter-add -> PReLU) on 8 TRN2 cores.

Strategy (graph-parallel over target nodes, replicated feature table):
  - Host: add self-loops, compute symmetric-norm dinv=1/sqrt(deg) and
    pre-scale x rows by dinv (so messages need no per-edge scale), sort
    edges by (target window, source parity, source row), split each
    window's edges into parity-uniform 128-edge blocks, and deal windows
    to cores sorted by block count so the shared static per-slot block
    shapes are tight (sorted-deal, ~5% fewer padded slots).
  - Device (SPMD, same program, per-core data): dma_gather the bf16
    source-row PAIRS of x_scaled (int16 pair indices, 256B elements,
    <=1024 indices per call) round-robined over 4 SWDGE queues so the
    four Q7 cpu-pairs generate descriptors concurrently (4x gather
    throughput vs a single queue).  Per 1024-slot call one batched DVE
    is_equal builds all eight 128x128 one-hot scatter matrices
    S[e,t] = (tgtlane[e] == t); each block is scatter-added via a PE
    matmul accumulating in PSUM:  agg[t,:] += sum_e S[e,t]*xs[src[e]].
    Self-loops use a static identity one-hot and the core's own rows.
    Then out.T[:,t] = W @ (dinv[t] * agg[t,:]) via PE transpose + matmul,
    add bias, PReLU, DMA out transposed.
  - Host: transpose + concatenate core outputs.
"""

import numpy as np
import ml_dtypes

N = 50000
E = 800000
D = 64
NCORES = 8
P = 128
TILES = 392                 # node tiles of 128 -> padded node count
NPAD = TILES * P            # 50176
WPC = TILES // NCORES       # 49 windows per core
OWN = WPC * P               # 6272 target nodes per core
CALL_BLOCKS = 8             # blocks (of 128 edges) per dma_gather call
CALL_IDX = CALL_BLOCKS * P  # 1024 indices per call (hw-safe limit)
NQUEUES = 4                 # SWDGE queues (disjoint Q7 cpu pairs)

_BF16 = ml_dtypes.bfloat16


def _host_prep(x, edge_index, W, b, prelu_a):
    rr = edge_index[0].astype(np.int64)
    cc = edge_index[1].astype(np.int64)

    # degree includes the self-loop (+1); self-loops are handled via a
    # dedicated per-window block fed from a static copy of the own x rows.
    deg = np.bincount(cc, minlength=NPAD).astype(np.float64) + 1.0
    dinv = (1.0 / np.sqrt(deg)).astype(np.float32)

    # sort by (window, parity of source): key = win * 2 + parity
    win = cc >> 7
    par = rr & 1
    key = win * 2 + par
    # within each (window, parity) run, sort by source row so each gather
    # call reads ascending DRAM addresses (better DRAM efficiency)
    order = np.lexsort((rr, key))
    rs = rr[order]
    cs = cc[order]
    ps = par[order]

    counts = np.bincount(key, minlength=TILES * 2).reshape(TILES, 2)
    starts = np.zeros(TILES * 2 + 1, np.int64)
    starts[1:] = np.cumsum(counts.reshape(-1))

    # sorted-deal window-to-core assignment: sort windows by their block
    # counts and deal groups of 8 across cores, so the (shared, static)
    # per-slot block counts are the max over near-identical windows
    # instead of the global max.  882 -> ~835 blocks per core.
    be = np.ceil(counts[:, 0] / P).astype(np.int64)
    bo = np.ceil(counts[:, 1] / P).astype(np.int64)
    worder = np.lexsort((bo, be))[::-1]
    assign = worder.reshape(WPC, NCORES)       # [slot, core] -> window
    BE = be[assign].max(axis=1)                # [WPC] even blocks per slot
    BO = bo[assign].max(axis=1)
    NBGs = BE + BO                             # gathered blocks per slot
    bprefix = np.zeros(WPC + 1, np.int64)
    bprefix[1:] = np.cumsum(NBGs)
    BG = int(bprefix[-1])        # gathered blocks per core
    NSLOT = BG * P               # gathered edge slots per core
    IDXC = NSLOT // 16           # wrapped idx columns

    # pre-scale x by dinv: messages become pure one-hot scatter-adds
    x_pad = np.zeros((NPAD, D), np.float32)
    x_pad[:N] = np.asarray(x, np.float32)
    x_sc = x_pad * dinv[:, None]
    x_bf = x_sc.astype(_BF16)
    x_pair = np.ascontiguousarray(x_bf.reshape(NPAD // 2, 2 * D))

    wt = np.ascontiguousarray(
        np.asarray(W, np.float32).T).astype(_BF16)              # [din, dout]
    b_col = np.asarray(b, np.float32).reshape(D, 1).copy()
    a_val = float(np.asarray(prelu_a).ravel()[0])
    iota_t = np.broadcast_to(
        np.arange(P, dtype=np.float32)[None, :], (P, P)
    ).astype(_BF16).copy()
    eye = np.eye(P, dtype=np.float32)
    eye_bf = eye.astype(_BF16)

    in_maps = []
    for k in range(NCORES):
        rows_slots = np.zeros(NSLOT, np.int64)
        coll_slots = np.full(NSLOT, 1000.0, np.float32)
        for i in range(WPC):
            w = int(assign[i, k])
            for p, nb, boff in ((0, BE[i], 0), (1, BO[i], BE[i])):
                s0, s1 = starts[2 * w + p], starts[2 * w + p + 1]
                base = (bprefix[i] + boff) * P
                nn = s1 - s0
                rows_slots[base:base + nn] = rs[s0:s1]
                coll_slots[base:base + nn] = (cs[s0:s1] & 127)
        coll_g = np.ascontiguousarray(
            coll_slots.reshape(BG, P).T.astype(_BF16))          # [P, BG]
        # gather indices: flat slot i holds srcrow//2, wrapped in 16
        # partitions and replicated across the 8 Q7 cores
        flat = (rows_slots >> 1).astype(np.int16)
        idxs = np.tile(flat.reshape(IDXC, 16).T, (8, 1))        # [128, IDXC]
        own_rows = assign[:, k]                                 # windows owned
        dinv_own = np.ascontiguousarray(
            dinv.reshape(TILES, P)[own_rows].T)                 # [P, WPC]
        # own (pre-scaled) x rows: x_own[p, 64i + c] = xs[win_i*128 + p, c]
        x_own = np.ascontiguousarray(
            x_bf.reshape(TILES, P, D)[own_rows]
            .transpose(1, 0, 2).reshape(P, WPC * D))
        in_maps.append({
            "x_pair": x_pair,
            "x_own": x_own,
            "idxs": np.ascontiguousarray(idxs),
            "coll_g": coll_g,
            "dinv_own": dinv_own,
            "w_t": wt,
            "b_col": b_col,
            "iota_t": iota_t,
            "eye": eye,
            "eye_bf": eye_bf,
        })
    meta = {"BE": [int(v) for v in BE], "BO": [int(v) for v in BO],
            "A": a_val, "assign": assign.tolist()}
    return in_maps, meta


def _unshard(results, meta):
    assign = np.asarray(meta["assign"])
    out = np.empty((NPAD, D), np.float32)
    for k in range(NCORES):
        o = results[k]["out_t"]
        for i in range(WPC):
            w = int(assign[i, k])
            out[w * P:(w + 1) * P] = o[:, i * P:(i + 1) * P].T
    return out[:N]


def _build_program(meta):
    import concourse.bacc as bacc
    import concourse.tile as tile
    import concourse.mybir as mybir

    dt = mybir.dt
    BE = meta["BE"]
    BO = meta["BO"]
    A_SCALE = meta["A"]
    NBGs = [be + bo for be, bo in zip(BE, BO)]
    bprefix = [0]
    for v in NBGs:
        bprefix.append(bprefix[-1] + v)
    BG = bprefix[-1]
    NSLOT = BG * P
    IDXC = NSLOT // 16

    nc = bacc.Bacc("TRN2", target_bir_lowering=False, debug=False,
                   num_devices=NCORES, num_swdge_queues=NQUEUES,
                   dynamic_dma_scratch_size=65536)
    x_pair = nc.dram_tensor("x_pair", [NPAD // 2, 2 * D], dt.bfloat16,
                            kind="ExternalInput")
    x_own = nc.dram_tensor("x_own", [P, WPC * D], dt.bfloat16,
                           kind="ExternalInput")
    idxs = nc.dram_tensor("idxs", [P, IDXC], dt.int16, kind="ExternalInput")
    coll_g = nc.dram_tensor("coll_g", [P, BG], dt.bfloat16,
                            kind="ExternalInput")
    dinv_own = nc.dram_tensor("dinv_own", [P, WPC], dt.float32,
                              kind="ExternalInput")
    eye = nc.dram_tensor("eye", [P, P], dt.float32, kind="ExternalInput")
    w_t = nc.dram_tensor("w_t", [D, D], dt.bfloat16, kind="ExternalInput")
    b_col = nc.dram_tensor("b_col", [D, 1], dt.float32, kind="ExternalInput")
    iota = nc.dram_tensor("iota_t", [P, P], dt.bfloat16, kind="ExternalInput")
    eye_bf = nc.dram_tensor("eye_bf", [P, P], dt.bfloat16,
                            kind="ExternalInput")
    out_t = nc.dram_tensor("out_t", [D, OWN], dt.float32, kind="ExternalOutput")

    with tile.TileContext(nc) as tc:
        CHUNK_A = min(8 * (CALL_IDX // 16), IDXC)   # first 8 calls' indices
        with (
            tc.tile_pool(name="const", bufs=1) as const,
            tc.tile_pool(name="xg", bufs=12) as xg,
            tc.tile_pool(name="sp", bufs=12) as sp,
            tc.tile_pool(name="work", bufs=8) as work,
            tc.tile_pool(name="psagg", bufs=4, space="PSUM") as psagg,
            tc.tile_pool(name="pst", bufs=2, space="PSUM") as pst,
            tc.tile_pool(name="pso", bufs=2, space="PSUM") as pso,
        ):
            # load the first calls' indices in a small tile, issued from
            # the Activation engine whose preamble finishes ~4us before the
            # Sync engine's, so the gather stream starts earlier
            idx_dummy = const.tile([P, 8], dt.int16)
            nc.gpsimd.memset(idx_dummy[:], 0)
            x_dummy = const.tile([P, P], dt.bfloat16)
            nc.gpsimd.dma_gather(
                x_dummy[:].rearrange("p (q e) -> p q e", e=P),
                x_pair[:], idx_dummy[:], P, P, P, queue_num=0)
            idx_a = const.tile([P, CHUNK_A], dt.int16)
            nc.scalar.dma_start(out=idx_a[:], in_=idxs[:, :CHUNK_A])
            idx_b = const.tile([P, IDXC - CHUNK_A], dt.int16)
            nc.sync.dma_start(out=idx_b[:], in_=idxs[:, CHUNK_A:])
            x_own_sb = const.tile([P, WPC * D], dt.bfloat16)
            nc.sync.dma_start(out=x_own_sb[:], in_=x_own[:])
            coll_sb = const.tile([P, BG], dt.bfloat16)
            nc.sync.dma_start(out=coll_sb[:], in_=coll_g[:])
            dinv_own_sb = const.tile([P, WPC], dt.float32)
            nc.sync.dma_start(out=dinv_own_sb[:], in_=dinv_own[:])
            eye_sb = const.tile([P, P], dt.float32)
            nc.sync.dma_start(out=eye_sb[:], in_=eye[:])
            wt_sb = const.tile([D, D], dt.bfloat16)
            nc.sync.dma_start(out=wt_sb[:], in_=w_t[:])
            b_sb = const.tile([D, 1], dt.float32)
            nc.sync.dma_start(out=b_sb[:], in_=b_col[:])
            iota_sb = const.tile([P, P], dt.bfloat16)
            nc.sync.dma_start(out=iota_sb[:], in_=iota[:])
            eyebf_sb = const.tile([P, P], dt.bfloat16)
            nc.sync.dma_start(out=eyebf_sb[:], in_=eye_bf[:])

            x_tiles = {}

            def gather_call(m):
                nblk = min(CALL_BLOCKS, BG - m * CALL_BLOCKS)
                ni = nblk * P
                X = xg.tile([P, CALL_BLOCKS * P], dt.bfloat16, tag="xg")
                c0 = m * (CALL_IDX // 16)
                if c0 + ni // 16 <= CHUNK_A:
                    idx_ap = idx_a[:, c0:c0 + ni // 16]
                else:
                    idx_ap = idx_b[:, c0 - CHUNK_A:c0 - CHUNK_A + ni // 16]
                nc.gpsimd.dma_gather(
                    X[:, :ni].rearrange("p (q e) -> p q e", e=P),
                    x_pair[:],
                    idx_ap,
                    ni,
                    ni,
                    P,  # elem_size (bf16 elems) = 256B = one row pair
                    queue_num=m % NQUEUES,
                )
                # batched one-hot build for all blocks of this call:
                # S[p, q*128 + t] = (iota[p, t] == coll_g[p, m*8 + q])
                S = sp.tile([P, CALL_BLOCKS * P], dt.bfloat16, tag="sp")
                nc.vector.tensor_tensor(
                    out=S[:, :ni].rearrange("p (q t) -> p q t", t=P),
                    in0=iota_sb[:, None, :].broadcast_to([P, nblk, P]),
                    in1=coll_sb[:, m * CALL_BLOCKS:m * CALL_BLOCKS + nblk,
                                None].broadcast_to([P, nblk, P]),
                    op=mybir.AluOpType.is_equal,
                )
                x_tiles[m] = (X, S)

            def emit_chain(w, agg_s):
                # transpose [P, D] -> [D, P]
                tp = pst.tile([D, P], dt.bfloat16, space="PSUM")
                nc.tensor.transpose(out=tp[:], in_=agg_s[:],
                                    identity=eyebf_sb[:])
                agg_tt = work.tile([D, P], dt.bfloat16, tag="aggt")
                nc.scalar.copy(out=agg_tt[:], in_=tp[:])
                # W @ aggT -> [D, P]
                o3 = pso.tile([D, P], dt.float32, space="PSUM")
                nc.tensor.matmul(out=o3[:], lhsT=wt_sb[:], rhs=agg_tt[:],
                                 start=True, stop=True)
                # native parametric relu: prelu(o3 + b) in one Scalar op
                ot = work.tile([D, P], dt.float32, tag="ot")
                nc.scalar.activation(
                    out=ot[:], in_=o3[:],
                    func=mybir.ActivationFunctionType.Prelu,
                    bias=b_sb[:, 0:1], scale=1.0, alpha=A_SCALE)
                nc.sync.dma_start(out=out_t[:, w * P:(w + 1) * P],
                                  in_=ot[:])

            pending = []
            for w in range(WPC):
                nbg_w = NBGs[w]
                agg_p = psagg.tile([P, D], dt.float32, space="PSUM")
                for j in range(nbg_w + 1):
                    if j < nbg_w:
                        bb = bprefix[w] + j
                        m, q = divmod(bb, CALL_BLOCKS)
                        if m not in x_tiles:
                            gather_call(m)
                        X, S = x_tiles[m]
                        h = 0 if j < BE[w] else D  # parity half of the pair
                        lhsT = S[:, q * P:(q + 1) * P]
                        rhs = X[:, q * P + h:q * P + h + D]
                    else:       # self-loop block: static identity one-hot
                        lhsT = eyebf_sb[:]
                        rhs = x_own_sb[:, w * D:(w + 1) * D]
                    nc.tensor.matmul(
                        out=agg_p[:], lhsT=lhsT, rhs=rhs,
                        start=(j == 0), stop=(j == nbg_w))

                # dinv[t] * agg fused into the PSUM -> SBUF copy on the
                # Scalar engine (keeps the in-order DVE queue free for
                # S-builds, which feed the gather-consuming matmuls)
                agg_s = work.tile([P, D], dt.bfloat16, tag="aggs")
                nc.scalar.activation(
                    out=agg_s[:], in_=agg_p[:],
                    func=mybir.ActivationFunctionType.Copy,
                    scale=dinv_own_sb[:, w:w + 1])
                # delay the PE transpose + W matmul by 2 windows so the
                # in-order PE never parks waiting on the Scalar copies
                # (which would stall the next window's scatter matmuls)
                pending.append((w, agg_s))
                if len(pending) > 2:
                    emit_chain(*pending.pop(0))
            for item in pending:
                emit_chain(*item)

    nc.compile()
    return nc


def kernel(x, edge_index, W, b, prelu_a):
    from concourse.bass_utils import run_bass_kernel_spmd

    in_maps, meta = _host_prep(x, edge_index, W, b, prelu_a)
    nc = _build_program(meta)
    res = run_bass_kernel_spmd(nc, in_maps, list(range(NCORES)))
    return _unshard(res.results, meta)

